# revision 32
# baseline (speedup 1.0000x reference)
"""MBD degradation-imputation sampling step on 8 Trainium2 NeuronCores.

v6 strategy (data-parallel over N=2048 candidates, 256/core), sample-major
tiles [128 samples, 1024 tf], chunk-major loop:

  pass A : ONE HBM pass over eps (~34 MB/core, the DMA roofline).  The
           c0/q' per-chunk rows are broadcast to 128 partitions by the
           TensorEngine (K=1 matmuls from a preloaded [32,2048] fp16
           tile) instead of 48 MB of partition_broadcast DMA (the v5
           bottleneck: 85 MB SBUF writes, DMA 72% busy).
           Engine split per [128,1024] tile:
             DVE    : u16 = eps + c0  (TT fp32+PSUM -> fp16, 1.04 ns/e)
                      v16 = clip(u16) (TS fp16 4x mode, 0.26 ns/e) -> vcache
             ACT    : Sa += sum(v16^2)     (Square + accum_out)
             GpSimd : Sb += sum(v16*q16)   (STT + accum_out, SBUF-only)
           score = cA*(Sa - 2*Sb) + sample-independent shift; observed
           positions saturate (c0=6e4 -> v=m, q=m) and cancel.
  softmax: single 1 KB AllGather of all 256 local scores, stats on the
           gathered 2048, un-normalized exp(); local Z rides slot TF of
           the AllReduce buffer.
  pass B : weighted partition-reduction from the fp16 vcache on the
           TensorEngine (M=1 fp16 matmuls, PSUM-accumulated), bounce
           split Vector/Scalar, AllReduce (T,F)+Z, final combine.

PSUM: one [128,1024] f32 tag rotating 3 buffers serves the c0/q
broadcasts in pass A and wrow/bps/qps (as slices) afterwards.

`stage` truncates for bisection: 1 = pass A only, 2 = +AG/softmax,
3 = +pass B (no AllReduce), 4 = full kernel.
"""

from contextlib import ExitStack

import numpy as np

import concourse.bass as bass
import concourse.tile as tile
from concourse import bacc, mybir
from concourse.bass_utils import run_bass_kernel_spmd

N_CORES = 8
N, T, F = 2048, 512, 64
P = 128
TF = T * F                      # 32768
NLOC = N // N_CORES             # 256
NBLK = NLOC // P                # 2
CHUNK = 1024
NCHUNK = TF // CHUNK            # 32
SUB = 512                       # matmul N (one PSUM bank)
TEMP = 0.1
T_STEPS = 1000
SAT = 60000.0                   # fp16-exact saturation for observed c0

F32 = mybir.dt.float32
F16 = mybir.dt.float16
AX = mybir.AxisListType
ALU = mybir.AluOpType
ACTF = mybir.ActivationFunctionType

# GpSimd tensor ops are BANNED from the hot loop: Pool TensorTensor
# activity poisons the DVE fast TS modes (clips measured 428ns clean
# vs 1410-2294ns while Pool TT runs), costing more DVE time than the
# offload saves.  All elementwise work runs DVE/ACT only.


def _schedule_scalars(i: int):
    s = 0.008
    x = np.linspace(0, T_STEPS, T_STEPS + 1, dtype=np.float64)
    ac = np.cos((x / T_STEPS + s) / (1 + s) * np.pi * 0.5) ** 2
    ac = ac / ac[0]
    betas = np.clip(1.0 - ac[1:] / ac[:-1], 0.0, 0.999)
    alphas = 1.0 - betas
    acp = np.cumprod(alphas)
    abar_i = np.float32(acp[i])
    sigma_i = np.float32(np.sqrt(1.0 - acp[i]))
    alpha_i = np.float32(alphas[i])
    abar_im1 = np.float32(acp[i - 1])
    sa = np.float32(np.sqrt(abar_i))
    # the reference's Yi terms cancel exactly; out_missing = c1 * weighted
    c1 = np.float32(sa / np.float32(np.sqrt(alpha_i)) / np.float32(np.sqrt(abar_im1)))
    return sigma_i, c1


def _build(sigma_i: float, c1: float, stage: int = 4):
    inv_sig = float(np.float32(1.0 / np.float32(sigma_i)))
    sigma_i = float(np.float32(sigma_i))
    c1 = float(np.float32(c1))
    # scores = cA * sum((v - q')^2)  (+ sample-independent shift vs ref)
    cA = float(np.float32(-(np.float32(sigma_i) ** 2) / np.float32(TF)))

    nc = bacc.Bacc(
        "TRN2", target_bir_lowering=False, debug=False, num_devices=N_CORES
    )
    eps_d = nc.dram_tensor("eps16", [NLOC, TF], F16, kind="ExternalInput")
    # cq16[k] = [c0 chunk k | q chunk k] fp16 — one broadcast per chunk
    cq_d = nc.dram_tensor("cq16", [NCHUNK, 2 * CHUNK], F16,
                          kind="ExternalInput")
    obs_d = nc.dram_tensor("obs", [TF], F32, kind="ExternalInput")
    maskf_d = nc.dram_tensor("maskf", [TF], F32, kind="ExternalInput")
    out_d = nc.dram_tensor("out", [TF], F32, kind="ExternalOutput")

    ones_d = nc.dram_tensor("ones", [P], F32, kind="ExternalInput")
    sc_loc_d = nc.dram_tensor("sc_loc", [NLOC], F32)
    sc_all_d = nc.dram_tensor("sc_all", [N], F32, addr_space="Shared")
    # ws carries the TF weighted partials plus the local softmax
    # normalizer Z in slot TF — one AllReduce delivers both.
    ws_loc_d = nc.dram_tensor("ws_loc", [TF + 4], F32)
    ws_all_d = nc.dram_tensor("ws_all", [TF + 4], F32, addr_space="Shared")

    rg = [list(range(N_CORES))]

    with tile.TileContext(nc) as tc, ExitStack() as ctx:
        eps_ap = eps_d.ap()

        rowsq = ctx.enter_context(tc.tile_pool(name="rowsq", bufs=1))
        work = ctx.enter_context(tc.tile_pool(name="work", bufs=3))
        cache = ctx.enter_context(tc.tile_pool(name="cache", bufs=1))
        stat = ctx.enter_context(tc.tile_pool(name="stat", bufs=1))
        smal = ctx.enter_context(tc.tile_pool(name="smal", bufs=1))
        psum = ctx.enter_context(tc.tile_pool(name="psum", bufs=3, space="PSUM"))

        def pstile(name):
            return psum.tile([P, CHUNK], F32, tag="ps", bufs=3, name=name)

        # fp16 clipped-values cache: 64 tiles of [128, 1024] packed into
        # one persistent tile (128 KiB per partition)
        vcache = cache.tile([P, NBLK * NCHUNK * CHUNK], F16, tag="vc",
                            name="vcache")

        # ---------------- pass A: local scores ----------------
        # All-fp16 SBUF elementwise ops keep every DVE instruction in a
        # fast mode (add/sub TT 2x_1p ~0.67 ns/e, clip TS ~0.42 ns/e).
        # c0/q broadcast tiles arrive via partition_broadcast DMA on the
        # gpsimd queue (Pool engine is otherwise idle — its TENSOR ops
        # are banned since they poison the DVE fast modes).
        sd_cols = [
            stat.tile([P, NCHUNK], F32, tag=f"sd{b}", name=f"sd_cols{b}")
            for b in range(NBLK)
        ]
        s_loc = stat.tile([P, NBLK], F32, tag="sloc", name="s_loc")
        # 4 KB partition lines everywhere: a single DMA queue caps near
        # 110 GB/s with 2 KB lines (measured), so eps comes in [128,2048]
        # double-chunk tiles on the sync queue and [c0|q] in one combined
        # [128,2048] broadcast per chunk on the gpsimd queue.
        # spread the ~33.6 MB of pass-A DMA over three queues (a single
        # queue caps near 112 GB/s): eps block0->sync, block1->scalar,
        # first bcast->gpsimd, second bcast rotates across all three.
        rot = [nc.sync, nc.scalar, nc.gpsimd]
        for kk in range(NCHUNK // 2):
            cqb = [None, None]
            for j in range(2):
                k = 2 * kk + j
                cqb[j] = rowsq.tile([P, 2 * CHUNK], F16, tag="cqb", bufs=3,
                                    name="cqb")
                eng = nc.gpsimd if j == 0 else rot[kk % 3]
                eng.dma_start(
                    out=cqb[j][:],
                    in_=cq_d.ap()[k:k + 1, :].partition_broadcast(P),
                )
            for b in range(NBLK):
                u_t = work.tile([P, 2 * CHUNK], F16, tag="u", bufs=3,
                                name="u_t")
                (nc.sync if b == 0 else nc.scalar).dma_start(
                    out=u_t[:],
                    in_=eps_ap[b * P:(b + 1) * P,
                               kk * 2 * CHUNK:(kk + 1) * 2 * CHUNK],
                )
                for j in range(2):
                    k = 2 * kk + j
                    c0_t = cqb[j][:, 0:CHUNK]
                    q16b = cqb[j][:, CHUNK:2 * CHUNK]
                    usl = u_t[:, j * CHUNK:(j + 1) * CHUNK]
                    u16 = work.tile([P, CHUNK], F16, tag="u16", bufs=2,
                                    name="u16")
                    nc.vector.tensor_tensor(u16[:], usl, c0_t, ALU.add)
                    off = (k * NBLK + b) * CHUNK
                    vsl = vcache[:, off:off + CHUNK]
                    nc.vector.tensor_scalar(
                        out=vsl, in0=u16[:], scalar1=inv_sig,
                        scalar2=-inv_sig, op0=ALU.min, op1=ALU.max,
                    )
                    d16 = work.tile([P, CHUNK], F16, tag="d16", bufs=3,
                                    name="d16")
                    nc.vector.tensor_tensor(d16[:], vsl, q16b, ALU.subtract)
                    d2 = work.tile([P, CHUNK], F16, tag="d2", bufs=2,
                                   name="d2")
                    nc.scalar.activation(
                        out=d2[:], in_=d16[:], func=ACTF.Square,
                        accum_out=sd_cols[b][:, k:k + 1],
                    )
        # score finalize: s = cA*sum(d^2)
        for b in range(NBLK):
            sd_tot = smal.tile([P, 1], F32, tag="sdt", name="sd_tot")
            nc.vector.tensor_reduce(sd_tot[:], sd_cols[b][:], axis=AX.X, op=ALU.add)
            nc.vector.tensor_scalar_mul(s_loc[:, b:b + 1], sd_tot[:], cA)
        nc.sync.dma_start(
            out=sc_loc_d.ap().rearrange("(b p) -> p b", b=NBLK),
            in_=s_loc[:],
        )
        if stage >= 2:
            nc.gpsimd.collective_compute(
                "AllGather", ALU.bypass,
                ins=[sc_loc_d.ap()],
                outs=[sc_all_d.ap()],
                replica_groups=rg,
            )
        if stage <= 1:
            nc.sync.dma_start(
                out=out_d.ap()[0:NLOC].rearrange("(b p) -> p b", p=P),
                in_=s_loc[:],
            )

        # ---------------- softmax stats ----------------
        # weights are UN-normalized exp(); the global Z rides the
        # AllReduce (slot TF of ws) and division happens post-reduce.
        wt16 = None
        if stage >= 2:
            onesr = smal.tile([1, P], F32, tag="onesr", name="onesr")
            nc.sync.dma_start(
                out=onesr[:], in_=ones_d.ap().rearrange("(a n) -> a n", a=1)
            )
            onec = smal.tile([P, 1], F32, tag="onec", name="onec")
            nc.sync.dma_start(
                out=onec[:], in_=ones_d.ap().rearrange("(p a) -> p a", a=1)
            )
            s_all = smal.tile([1, N], F32, tag="sall", name="s_all")
            nc.sync.dma_start(
                out=s_all[:], in_=sc_all_d.ap().rearrange("(a n) -> a n", a=1)
            )
            pack = smal.tile([1, 2], F32, tag="pack", name="pack")
            negmean = smal.tile([1, 1], F32, tag="negmean", name="negmean")
            nc.vector.tensor_reduce(negmean[:], s_all[:], axis=AX.X, op=ALU.add)
            nc.vector.tensor_scalar_mul(negmean[:], negmean[:], -1.0 / N)
            js = smal.tile([1, N], F16, tag="js", name="js")
            ssq = smal.tile([1, 1], F32, tag="ssq", name="ssq")
            nc.scalar.activation(
                out=js[:], in_=s_all[:], func=ACTF.Square, bias=negmean[:],
                accum_out=ssq[:],
            )
            # std = max(sqrt(ssq/(N-1)), 1e-4); pack0 = 1/(std*TEMP)
            std = smal.tile([1, 1], F32, tag="std", name="std")
            nc.scalar.activation(
                out=std[:], in_=ssq[:], func=ACTF.Sqrt, scale=1.0 / (N - 1)
            )
            stdT = smal.tile([1, 1], F32, tag="stdT", name="stdT")
            nc.vector.tensor_scalar(
                out=stdT[:], in0=std[:], scalar1=1e-4, scalar2=TEMP,
                op0=ALU.max, op1=ALU.mult,
            )
            nc.vector.reciprocal(pack[:, 0:1], stdT[:])
            mx = smal.tile([1, 1], F32, tag="mx", name="mx")
            nc.vector.tensor_reduce(mx[:], s_all[:], axis=AX.X, op=ALU.max)
            # shifted logit: (s - mx)*inv10 (mean cancels in the shift, and
            # the un-normalized exp is safe: max exponent is exactly 0)
            nmx = smal.tile([1, 1], F32, tag="nmx", name="nmx")
            nc.vector.tensor_scalar_mul(nmx[:], mx[:], -1.0)
            nc.vector.tensor_tensor(pack[:, 1:2], nmx[:], pack[:, 0:1], ALU.mult)
            # PE-broadcast (inv10, bg) to all 128 partitions
            bps = pstile("bps")
            nc.tensor.matmul(bps[:, 0:2], lhsT=onesr[:], rhs=pack[:],
                             start=True, stop=True)
            scal = smal.tile([P, 2], F32, tag="scal", name="scal")
            nc.vector.tensor_copy(scal[:], bps[:, 0:2])

            # warm the PE p-state before pass B: back-to-back [1,512] dummy
            # matmuls gated on the post-stats scal tile keep the PE busy
            # >3us so the real fp16 matmuls run at 2.4 GHz, not 1.2
            jl = smal.tile([P, 1], F16, tag="jl", name="jl")
            nc.scalar.copy(jl[:], scal[:, 0:1])
            for w in range(20):
                wmm = pstile("wmm")
                nc.tensor.matmul(wmm[0:1, 0:SUB], lhsT=jl[:],
                                 rhs=vcache[:, 0:SUB], start=True, stop=True)

            e_loc = smal.tile([P, NBLK], F32, tag="eloc", name="e_loc")
            nc.scalar.activation(
                out=e_loc[:], in_=s_loc[:], func=ACTF.Exp,
                scale=scal[:, 0:1], bias=scal[:, 1:2],
            )
            wt16 = stat.tile([P, NBLK], F16, tag="wt16", name="wt16")
            zloc = smal.tile([P, 1], F32, tag="zloc", name="zloc")
            nc.scalar.activation(
                out=wt16[:], in_=e_loc[:], func=ACTF.Copy, accum_out=zloc[:]
            )
            # local Z -> ws_loc[TF] so the AllReduce sums it globally
            zpt = pstile("zpt")
            zps = zpt[0:1, 0:1]
            nc.tensor.matmul(zps, lhsT=zloc[:], rhs=onec[:], start=True,
                             stop=True)
            ztot = smal.tile([1, 1], F32, tag="ztot", name="ztot")
            nc.vector.tensor_copy(ztot[:], zps)
            nc.sync.dma_start(
                out=ws_loc_d.ap()[TF:TF + 1].rearrange("(a n) -> a n", a=1),
                in_=ztot[:],
            )
            if stage <= 2:
                nc.sync.dma_start(
                    out=out_d.ap()[0:NLOC].rearrange("(b p) -> p b", p=P),
                    in_=e_loc[:],
                )

        # ---------------- pass B: weighted sum on PE from SBUF cache ----
        if stage >= 3:
            # two 512-wide PSUM rows (= one 1024 chunk) per bounce tile:
            # halves the copy and writeback-DMA count.  The AllReduce is
            # split in two so the first half overlaps the second half's
            # matmuls.
            for k in range(NCHUNK):
                wrow = pstile("wrow")
                for half in range(2):
                    for b in range(NBLK):
                        off = (k * NBLK + b) * CHUNK + half * SUB
                        nc.tensor.matmul(
                            wrow[0:1, half * SUB:(half + 1) * SUB],
                            lhsT=wt16[:, b:b + 1],
                            rhs=vcache[:, off:off + SUB],
                            start=(b == 0), stop=(b == NBLK - 1),
                        )
                wsb = work.tile([1, CHUNK], F32, tag="wsb", bufs=3, name="wsb")
                if k % 2 == 0:
                    nc.vector.tensor_copy(wsb[:], wrow[0:1, :])
                else:
                    nc.scalar.copy(wsb[:], wrow[0:1, :])
                nc.sync.dma_start(
                    out=ws_loc_d.ap()[k * CHUNK:(k + 1) * CHUNK]
                    .rearrange("(a n) -> a n", a=1),
                    in_=wsb[:],
                )
                if stage >= 4 and k == NCHUNK // 2 - 1:
                    nc.gpsimd.collective_compute(
                        "AllReduce", ALU.add,
                        ins=[ws_loc_d.ap()[0:TF // 2]],
                        outs=[ws_all_d.ap()[0:TF // 2]],
                        replica_groups=rg,
                    )
            if stage <= 3:
                o3 = stat.tile([P, TF // P], F32, tag="o3", name="o3")
                nc.sync.dma_start(
                    out=o3[:],
                    in_=ws_loc_d.ap()[0:TF].rearrange("(p c) -> p c", p=P),
                )
                nc.sync.dma_start(
                    out=out_d.ap().rearrange("(p c) -> p c", p=P), in_=o3[:]
                )

        # ---------------- AllReduce (2nd half) + final combine ----------
        # obs_d carries mask*obs and maskf_d carries (1-mask) so the
        # final combine is out = (w*qb)*m1 + mobs: 3 vector ops.
        if stage >= 4:
            rowmaj0 = lambda d: d.ap()[0:TF].rearrange("(p c) -> p c", p=P)
            obs_t = stat.tile([P, TF // P], F32, tag="obsf", name="obs_t")
            nc.sync.dma_start(out=obs_t[:], in_=rowmaj0(obs_d))
            m_t = stat.tile([P, TF // P], F32, tag="mf", name="m_t")
            nc.sync.dma_start(out=m_t[:], in_=rowmaj0(maskf_d))
            nc.gpsimd.collective_compute(
                "AllReduce", ALU.add,
                ins=[ws_loc_d.ap()[TF // 2:TF + 4]],
                outs=[ws_all_d.ap()[TF // 2:TF + 4]],
                replica_groups=rg,
            )
            w_t = stat.tile([P, TF // P], F32, tag="wfin", name="w_t")
            nc.sync.dma_start(out=w_t[:], in_=rowmaj0(ws_all_d))
            zg = smal.tile([1, 1], F32, tag="zg", name="zg")
            nc.sync.dma_start(
                out=zg[:],
                in_=ws_all_d.ap()[TF:TF + 1].rearrange("(a n) -> a n", a=1),
            )
            rzg = smal.tile([1, 1], F32, tag="rzg", name="rzg")
            nc.vector.reciprocal(rzg[:], zg[:])
            qfin = smal.tile([1, 1], F32, tag="qfin", name="qfin")
            nc.vector.tensor_scalar_mul(qfin[:], rzg[:], float(c1 * sigma_i))
            qps = pstile("qps")
            nc.tensor.matmul(qps[:, 0:1], lhsT=onesr[:], rhs=qfin[:],
                             start=True, stop=True)
            qb = smal.tile([P, 1], F32, tag="qb", name="qb")
            nc.vector.tensor_copy(qb[:], qps[:, 0:1])
            t1 = stat.tile([P, TF // P], F32, tag="t1", name="t1")
            nc.vector.tensor_single_scalar(
                out=t1[:], in_=w_t[:], scalar=qb[:], op=ALU.mult
            )
            # out = t1*m1 + mobs
            nc.vector.tensor_tensor(t1[:], t1[:], m_t[:], ALU.mult)
            nc.vector.tensor_tensor(t1[:], t1[:], obs_t[:], ALU.add)
            nc.sync.dma_start(out=rowmaj0(out_d), in_=t1[:])

    nc.compile()
    return nc


_CACHE: dict = {}
TRACE = False
STAGE = 4
LAST_RESULTS = None


def kernel(Xbar_i, observed_data, time_points, mask, eps, deg_a, deg_b, i):
    global LAST_RESULTS
    i = int(i)
    sigma_i, c1 = _schedule_scalars(i)
    key = ("v12", i, STAGE)
    if key not in _CACHE:
        _CACHE[key] = _build(float(sigma_i), float(c1), stage=STAGE)
    nc = _CACHE[key]

    inv_sig = np.float32(1.0) / sigma_i
    Xb = np.asarray(Xbar_i, np.float32)
    obs = np.asarray(observed_data, np.float32)
    msk = np.asarray(mask, bool)
    tp = np.asarray(time_points, np.float32)
    da = np.asarray(deg_a, np.float32)
    db = np.asarray(deg_b, np.float32)
    epsf = np.asarray(eps, np.float32)

    pred = da[None, :] + db[None, :] * tp[:, None]
    c0 = (Xb * inv_sig).astype(np.float32)
    c0 = np.where(msk, np.float32(SAT), c0).reshape(-1)
    qp = (pred * inv_sig).astype(np.float32)
    qp = np.where(msk, inv_sig, qp).reshape(-1)
    cq16 = np.concatenate(
        [c0.reshape(NCHUNK, CHUNK), qp.reshape(NCHUNK, CHUNK)], axis=1
    ).astype(np.float16)
    # final combine reads mask*obs and (1-mask) directly
    obsf = np.where(msk, obs, np.float32(0.0)).reshape(-1).astype(np.float32)
    maskf = (~msk).astype(np.float32).reshape(-1)

    eps16 = epsf.reshape(N, TF).astype(np.float16)
    in_maps = []
    for c in range(N_CORES):
        shard = np.ascontiguousarray(eps16[c * NLOC:(c + 1) * NLOC])
        in_maps.append(
            {"eps16": shard, "cq16": cq16, "obs": obsf,
             "maskf": maskf, "ones": np.ones(P, np.float32)}
        )
    kr = run_bass_kernel_spmd(nc, in_maps, list(range(N_CORES)), trace=TRACE)
    LAST_RESULTS = kr
    return kr.results[0]["out"].reshape(T, F).astype(np.float32)


# revision 35
# speedup vs baseline: 1.1290x; 1.1290x over previous
"""MBD degradation-imputation sampling step on 8 Trainium2 NeuronCores.

v6 strategy (data-parallel over N=2048 candidates, 256/core), sample-major
tiles [128 samples, 1024 tf], chunk-major loop:

  pass A : ONE HBM pass over eps (~34 MB/core, the DMA roofline).  The
           c0/q' per-chunk rows are broadcast to 128 partitions by the
           TensorEngine (K=1 matmuls from a preloaded [32,2048] fp16
           tile) instead of 48 MB of partition_broadcast DMA (the v5
           bottleneck: 85 MB SBUF writes, DMA 72% busy).
           Engine split per [128,1024] tile:
             DVE    : u16 = eps + c0  (TT fp32+PSUM -> fp16, 1.04 ns/e)
                      v16 = clip(u16) (TS fp16 4x mode, 0.26 ns/e) -> vcache
             ACT    : Sa += sum(v16^2)     (Square + accum_out)
             GpSimd : Sb += sum(v16*q16)   (STT + accum_out, SBUF-only)
           score = cA*(Sa - 2*Sb) + sample-independent shift; observed
           positions saturate (c0=6e4 -> v=m, q=m) and cancel.
  softmax: single 1 KB AllGather of all 256 local scores, stats on the
           gathered 2048, un-normalized exp(); local Z rides slot TF of
           the AllReduce buffer.
  pass B : weighted partition-reduction from the fp16 vcache on the
           TensorEngine (M=1 fp16 matmuls, PSUM-accumulated), bounce
           split Vector/Scalar, AllReduce (T,F)+Z, final combine.

PSUM: one [128,1024] f32 tag rotating 3 buffers serves the c0/q
broadcasts in pass A and wrow/bps/qps (as slices) afterwards.

`stage` truncates for bisection: 1 = pass A only, 2 = +AG/softmax,
3 = +pass B (no AllReduce), 4 = full kernel.
"""

from contextlib import ExitStack

import numpy as np

import concourse.bass as bass
import concourse.tile as tile
from concourse import bacc, mybir
from concourse.bass_utils import run_bass_kernel_spmd

N_CORES = 8
N, T, F = 2048, 512, 64
P = 128
TF = T * F                      # 32768
NLOC = N // N_CORES             # 256
NBLK = NLOC // P                # 2
CHUNK = 1024
NCHUNK = TF // CHUNK            # 32
SUB = 512                       # matmul N (one PSUM bank)
TEMP = 0.1
T_STEPS = 1000
SAT = 60000.0                   # fp16-exact saturation for observed c0

F32 = mybir.dt.float32
F16 = mybir.dt.float16
AX = mybir.AxisListType
ALU = mybir.AluOpType
ACTF = mybir.ActivationFunctionType

# GpSimd tensor ops are BANNED from the hot loop: Pool TensorTensor
# activity poisons the DVE fast TS modes (clips measured 428ns clean
# vs 1410-2294ns while Pool TT runs), costing more DVE time than the
# offload saves.  All elementwise work runs DVE/ACT only.


def _schedule_scalars(i: int):
    s = 0.008
    x = np.linspace(0, T_STEPS, T_STEPS + 1, dtype=np.float64)
    ac = np.cos((x / T_STEPS + s) / (1 + s) * np.pi * 0.5) ** 2
    ac = ac / ac[0]
    betas = np.clip(1.0 - ac[1:] / ac[:-1], 0.0, 0.999)
    alphas = 1.0 - betas
    acp = np.cumprod(alphas)
    abar_i = np.float32(acp[i])
    sigma_i = np.float32(np.sqrt(1.0 - acp[i]))
    alpha_i = np.float32(alphas[i])
    abar_im1 = np.float32(acp[i - 1])
    sa = np.float32(np.sqrt(abar_i))
    # the reference's Yi terms cancel exactly; out_missing = c1 * weighted
    c1 = np.float32(sa / np.float32(np.sqrt(alpha_i)) / np.float32(np.sqrt(abar_im1)))
    return sigma_i, c1


def _build(sigma_i: float, c1: float, stage: int = 4):
    inv_sig = float(np.float32(1.0 / np.float32(sigma_i)))
    sigma_i = float(np.float32(sigma_i))
    c1 = float(np.float32(c1))
    # scores = cA * sum((v - q')^2)  (+ sample-independent shift vs ref)
    cA = float(np.float32(-(np.float32(sigma_i) ** 2) / np.float32(TF)))

    nc = bacc.Bacc(
        "TRN2", target_bir_lowering=False, debug=False, num_devices=N_CORES
    )
    eps_d = nc.dram_tensor("eps16", [NLOC, TF], F16, kind="ExternalInput")
    c0_d = nc.dram_tensor("c016", [TF], F16, kind="ExternalInput")
    q_d = nc.dram_tensor("q16", [TF], F16, kind="ExternalInput")
    obs_d = nc.dram_tensor("obs", [TF], F32, kind="ExternalInput")
    maskf_d = nc.dram_tensor("maskf", [TF], F32, kind="ExternalInput")
    out_d = nc.dram_tensor("out", [TF], F32, kind="ExternalOutput")

    ones_d = nc.dram_tensor("ones", [P], F32, kind="ExternalInput")
    sc_loc_d = nc.dram_tensor("sc_loc", [NLOC], F32)
    sc_all_d = nc.dram_tensor("sc_all", [N], F32, addr_space="Shared")
    # ws carries the TF weighted partials plus the local softmax
    # normalizer Z in slot TF — one AllReduce delivers both.
    ws_loc_d = nc.dram_tensor("ws_loc", [TF + 4], F32)
    ws_all_d = nc.dram_tensor("ws_all", [TF + 4], F32, addr_space="Shared")

    rg = [list(range(N_CORES))]

    with tile.TileContext(nc) as tc, ExitStack() as ctx:
        eps_ap = eps_d.ap()

        rowsq = ctx.enter_context(tc.tile_pool(name="rowsq", bufs=1))
        work = ctx.enter_context(tc.tile_pool(name="work", bufs=3))
        cache = ctx.enter_context(tc.tile_pool(name="cache", bufs=1))
        stat = ctx.enter_context(tc.tile_pool(name="stat", bufs=1))
        smal = ctx.enter_context(tc.tile_pool(name="smal", bufs=1))
        psum = ctx.enter_context(tc.tile_pool(name="psum", bufs=3, space="PSUM"))

        def pstile(name):
            return psum.tile([P, CHUNK], F32, tag="ps", bufs=3, name=name)

        # fp16 clipped-values cache: 64 tiles of [128, 1024] packed into
        # one persistent tile (128 KiB per partition)
        vcache = cache.tile([P, NBLK * NCHUNK * CHUNK], F16, tag="vc",
                            name="vcache")

        # ---------------- pass A: local scores ----------------
        # All-fp16 SBUF elementwise ops keep every DVE instruction in a
        # fast mode (add/sub TT 2x_1p ~0.67 ns/e, clip TS ~0.42 ns/e).
        # c0/q broadcast tiles arrive via partition_broadcast DMA on the
        # gpsimd queue (Pool engine is otherwise idle — its TENSOR ops
        # are banned since they poison the DVE fast modes).
        sd_cols = [
            stat.tile([P, NCHUNK], F32, tag=f"sd{b}", name=f"sd_cols{b}")
            for b in range(NBLK)
        ]
        s_loc = stat.tile([P, NBLK], F32, tag="sloc", name="s_loc")
        # 4 KB partition lines everywhere: a single DMA queue caps near
        # 110 GB/s with 2 KB lines (measured), so eps comes in [128,2048]
        # double-chunk tiles on the sync queue and [c0|q] in one combined
        # [128,2048] broadcast per chunk on the gpsimd queue.
        # spread the ~33.6 MB of pass-A DMA over three queues (a single
        # queue caps near 112 GB/s): eps block0->sync, block1->scalar,
        # first bcast->gpsimd, second bcast rotates across all three.
        rot = [nc.sync, nc.scalar, nc.gpsimd]
        for kk in range(NCHUNK // 2):
            cqb = [None, None]
            for j in range(2):
                k = 2 * kk + j
                sl = slice(k * CHUNK, (k + 1) * CHUNK)
                cqb[j] = rowsq.tile([P, 2 * CHUNK], F16, tag="cqb", bufs=3,
                                    name="cqb")
                nc.gpsimd.dma_start(
                    out=cqb[j][:, 0:CHUNK],
                    in_=c0_d.ap()[sl].partition_broadcast(P),
                )
                rot[kk % 3].dma_start(
                    out=cqb[j][:, CHUNK:2 * CHUNK],
                    in_=q_d.ap()[sl].partition_broadcast(P),
                )
            for b in range(NBLK):
                u_t = work.tile([P, 2 * CHUNK], F16, tag="u", bufs=3,
                                name="u_t")
                (nc.sync if b == 0 else nc.scalar).dma_start(
                    out=u_t[:],
                    in_=eps_ap[b * P:(b + 1) * P,
                               kk * 2 * CHUNK:(kk + 1) * 2 * CHUNK],
                )
                for j in range(2):
                    k = 2 * kk + j
                    c0_t = cqb[j][:, 0:CHUNK]
                    q16b = cqb[j][:, CHUNK:2 * CHUNK]
                    usl = u_t[:, j * CHUNK:(j + 1) * CHUNK]
                    u16 = work.tile([P, CHUNK], F16, tag="u16", bufs=2,
                                    name="u16")
                    nc.vector.tensor_tensor(u16[:], usl, c0_t, ALU.add)
                    off = (k * NBLK + b) * CHUNK
                    vsl = vcache[:, off:off + CHUNK]
                    nc.vector.tensor_scalar(
                        out=vsl, in0=u16[:], scalar1=inv_sig,
                        scalar2=-inv_sig, op0=ALU.min, op1=ALU.max,
                    )
                    d16 = work.tile([P, CHUNK], F16, tag="d16", bufs=3,
                                    name="d16")
                    nc.vector.tensor_tensor(d16[:], vsl, q16b, ALU.subtract)
                    d2 = work.tile([P, CHUNK], F16, tag="d2", bufs=2,
                                   name="d2")
                    nc.scalar.activation(
                        out=d2[:], in_=d16[:], func=ACTF.Square,
                        accum_out=sd_cols[b][:, k:k + 1],
                    )
        # score finalize: s = cA*sum(d^2)
        for b in range(NBLK):
            sd_tot = smal.tile([P, 1], F32, tag="sdt", name="sd_tot")
            nc.vector.tensor_reduce(sd_tot[:], sd_cols[b][:], axis=AX.X, op=ALU.add)
            nc.vector.tensor_scalar_mul(s_loc[:, b:b + 1], sd_tot[:], cA)
        nc.sync.dma_start(
            out=sc_loc_d.ap().rearrange("(b p) -> p b", b=NBLK),
            in_=s_loc[:],
        )
        if stage >= 2:
            nc.gpsimd.collective_compute(
                "AllGather", ALU.bypass,
                ins=[sc_loc_d.ap()],
                outs=[sc_all_d.ap()],
                replica_groups=rg,
            )
        if stage <= 1:
            nc.sync.dma_start(
                out=out_d.ap()[0:NLOC].rearrange("(b p) -> p b", p=P),
                in_=s_loc[:],
            )

        # ---------------- softmax stats ----------------
        # weights are UN-normalized exp(); the global Z rides the
        # AllReduce (slot TF of ws) and division happens post-reduce.
        wt16 = None
        if stage >= 2:
            onesr = smal.tile([1, P], F32, tag="onesr", name="onesr")
            nc.sync.dma_start(
                out=onesr[:], in_=ones_d.ap().rearrange("(a n) -> a n", a=1)
            )
            onec = smal.tile([P, 1], F32, tag="onec", name="onec")
            nc.sync.dma_start(
                out=onec[:], in_=ones_d.ap().rearrange("(p a) -> p a", a=1)
            )
            s_all = smal.tile([1, N], F32, tag="sall", name="s_all")
            nc.sync.dma_start(
                out=s_all[:], in_=sc_all_d.ap().rearrange("(a n) -> a n", a=1)
            )
            pack = smal.tile([1, 2], F32, tag="pack", name="pack")
            negmean = smal.tile([1, 1], F32, tag="negmean", name="negmean")
            nc.vector.tensor_reduce(negmean[:], s_all[:], axis=AX.X, op=ALU.add)
            nc.vector.tensor_scalar_mul(negmean[:], negmean[:], -1.0 / N)
            js = smal.tile([1, N], F16, tag="js", name="js")
            ssq = smal.tile([1, 1], F32, tag="ssq", name="ssq")
            nc.scalar.activation(
                out=js[:], in_=s_all[:], func=ACTF.Square, bias=negmean[:],
                accum_out=ssq[:],
            )
            # std = max(sqrt(ssq/(N-1)), 1e-4); pack0 = 1/(std*TEMP)
            std = smal.tile([1, 1], F32, tag="std", name="std")
            nc.scalar.activation(
                out=std[:], in_=ssq[:], func=ACTF.Sqrt, scale=1.0 / (N - 1)
            )
            stdT = smal.tile([1, 1], F32, tag="stdT", name="stdT")
            nc.vector.tensor_scalar(
                out=stdT[:], in0=std[:], scalar1=1e-4, scalar2=TEMP,
                op0=ALU.max, op1=ALU.mult,
            )
            nc.vector.reciprocal(pack[:, 0:1], stdT[:])
            mx = smal.tile([1, 1], F32, tag="mx", name="mx")
            nc.vector.tensor_reduce(mx[:], s_all[:], axis=AX.X, op=ALU.max)
            # shifted logit: (s - mx)*inv10 (mean cancels in the shift, and
            # the un-normalized exp is safe: max exponent is exactly 0)
            nmx = smal.tile([1, 1], F32, tag="nmx", name="nmx")
            nc.vector.tensor_scalar_mul(nmx[:], mx[:], -1.0)
            nc.vector.tensor_tensor(pack[:, 1:2], nmx[:], pack[:, 0:1], ALU.mult)
            # PE-broadcast (inv10, bg) to all 128 partitions
            bps = pstile("bps")
            nc.tensor.matmul(bps[:, 0:2], lhsT=onesr[:], rhs=pack[:],
                             start=True, stop=True)
            scal = smal.tile([P, 2], F32, tag="scal", name="scal")
            nc.vector.tensor_copy(scal[:], bps[:, 0:2])

            # warm the PE p-state before pass B: back-to-back [1,512] dummy
            # matmuls gated on the post-stats scal tile keep the PE busy
            # >3us so the real fp16 matmuls run at 2.4 GHz, not 1.2
            jl = smal.tile([P, 1], F16, tag="jl", name="jl")
            nc.scalar.copy(jl[:], scal[:, 0:1])
            for w in range(20):
                wmm = pstile("wmm")
                nc.tensor.matmul(wmm[0:1, 0:SUB], lhsT=jl[:],
                                 rhs=vcache[:, 0:SUB], start=True, stop=True)

            e_loc = smal.tile([P, NBLK], F32, tag="eloc", name="e_loc")
            nc.scalar.activation(
                out=e_loc[:], in_=s_loc[:], func=ACTF.Exp,
                scale=scal[:, 0:1], bias=scal[:, 1:2],
            )
            wt16 = stat.tile([P, NBLK], F16, tag="wt16", name="wt16")
            zloc = smal.tile([P, 1], F32, tag="zloc", name="zloc")
            nc.scalar.activation(
                out=wt16[:], in_=e_loc[:], func=ACTF.Copy, accum_out=zloc[:]
            )
            # local Z -> ws_loc[TF] so the AllReduce sums it globally
            zpt = pstile("zpt")
            zps = zpt[0:1, 0:1]
            nc.tensor.matmul(zps, lhsT=zloc[:], rhs=onec[:], start=True,
                             stop=True)
            ztot = smal.tile([1, 1], F32, tag="ztot", name="ztot")
            nc.vector.tensor_copy(ztot[:], zps)
            nc.sync.dma_start(
                out=ws_loc_d.ap()[TF:TF + 1].rearrange("(a n) -> a n", a=1),
                in_=ztot[:],
            )
            if stage <= 2:
                nc.sync.dma_start(
                    out=out_d.ap()[0:NLOC].rearrange("(b p) -> p b", p=P),
                    in_=e_loc[:],
                )

        # ---------------- pass B: weighted sum on PE from SBUF cache ----
        if stage >= 3:
            # two 512-wide PSUM rows (= one 1024 chunk) per bounce tile:
            # halves the copy and writeback-DMA count.  The AllReduce is
            # split in two so the first half overlaps the second half's
            # matmuls.
            for k in range(NCHUNK):
                wrow = pstile("wrow")
                for half in range(2):
                    for b in range(NBLK):
                        off = (k * NBLK + b) * CHUNK + half * SUB
                        nc.tensor.matmul(
                            wrow[0:1, half * SUB:(half + 1) * SUB],
                            lhsT=wt16[:, b:b + 1],
                            rhs=vcache[:, off:off + SUB],
                            start=(b == 0), stop=(b == NBLK - 1),
                        )
                wsb = work.tile([1, CHUNK], F32, tag="wsb", bufs=3, name="wsb")
                if k % 2 == 0:
                    nc.vector.tensor_copy(wsb[:], wrow[0:1, :])
                else:
                    nc.scalar.copy(wsb[:], wrow[0:1, :])
                nc.sync.dma_start(
                    out=ws_loc_d.ap()[k * CHUNK:(k + 1) * CHUNK]
                    .rearrange("(a n) -> a n", a=1),
                    in_=wsb[:],
                )
                if stage >= 4 and k == NCHUNK // 2 - 1:
                    nc.gpsimd.collective_compute(
                        "AllReduce", ALU.add,
                        ins=[ws_loc_d.ap()[0:TF // 2]],
                        outs=[ws_all_d.ap()[0:TF // 2]],
                        replica_groups=rg,
                    )
            if stage <= 3:
                o3 = stat.tile([P, TF // P], F32, tag="o3", name="o3")
                nc.sync.dma_start(
                    out=o3[:],
                    in_=ws_loc_d.ap()[0:TF].rearrange("(p c) -> p c", p=P),
                )
                nc.sync.dma_start(
                    out=out_d.ap().rearrange("(p c) -> p c", p=P), in_=o3[:]
                )

        # ---------------- AllReduce (2nd half) + final combine ----------
        # obs_d carries mask*obs and maskf_d carries (1-mask) so the
        # final combine is out = (w*qb)*m1 + mobs: 3 vector ops.
        if stage >= 4:
            rowmaj0 = lambda d: d.ap()[0:TF].rearrange("(p c) -> p c", p=P)
            obs_t = stat.tile([P, TF // P], F32, tag="obsf", name="obs_t")
            nc.sync.dma_start(out=obs_t[:], in_=rowmaj0(obs_d))
            m_t = stat.tile([P, TF // P], F32, tag="mf", name="m_t")
            nc.sync.dma_start(out=m_t[:], in_=rowmaj0(maskf_d))
            nc.gpsimd.collective_compute(
                "AllReduce", ALU.add,
                ins=[ws_loc_d.ap()[TF // 2:TF + 4]],
                outs=[ws_all_d.ap()[TF // 2:TF + 4]],
                replica_groups=rg,
            )
            w_t = stat.tile([P, TF // P], F32, tag="wfin", name="w_t")
            nc.sync.dma_start(out=w_t[:], in_=rowmaj0(ws_all_d))
            zg = smal.tile([1, 1], F32, tag="zg", name="zg")
            nc.sync.dma_start(
                out=zg[:],
                in_=ws_all_d.ap()[TF:TF + 1].rearrange("(a n) -> a n", a=1),
            )
            rzg = smal.tile([1, 1], F32, tag="rzg", name="rzg")
            nc.vector.reciprocal(rzg[:], zg[:])
            qfin = smal.tile([1, 1], F32, tag="qfin", name="qfin")
            nc.vector.tensor_scalar_mul(qfin[:], rzg[:], float(c1 * sigma_i))
            qps = pstile("qps")
            nc.tensor.matmul(qps[:, 0:1], lhsT=onesr[:], rhs=qfin[:],
                             start=True, stop=True)
            qb = smal.tile([P, 1], F32, tag="qb", name="qb")
            nc.vector.tensor_copy(qb[:], qps[:, 0:1])
            t1 = stat.tile([P, TF // P], F32, tag="t1", name="t1")
            nc.vector.tensor_single_scalar(
                out=t1[:], in_=w_t[:], scalar=qb[:], op=ALU.mult
            )
            # out = t1*m1 + mobs
            nc.vector.tensor_tensor(t1[:], t1[:], m_t[:], ALU.mult)
            nc.vector.tensor_tensor(t1[:], t1[:], obs_t[:], ALU.add)
            nc.sync.dma_start(out=rowmaj0(out_d), in_=t1[:])

    nc.compile()
    return nc


_CACHE: dict = {}
TRACE = False
STAGE = 4
LAST_RESULTS = None


def kernel(Xbar_i, observed_data, time_points, mask, eps, deg_a, deg_b, i):
    global LAST_RESULTS
    i = int(i)
    sigma_i, c1 = _schedule_scalars(i)
    key = ("v13", i, STAGE)
    if key not in _CACHE:
        _CACHE[key] = _build(float(sigma_i), float(c1), stage=STAGE)
    nc = _CACHE[key]

    inv_sig = np.float32(1.0) / sigma_i
    Xb = np.asarray(Xbar_i, np.float32)
    obs = np.asarray(observed_data, np.float32)
    msk = np.asarray(mask, bool)
    tp = np.asarray(time_points, np.float32)
    da = np.asarray(deg_a, np.float32)
    db = np.asarray(deg_b, np.float32)
    epsf = np.asarray(eps, np.float32)

    pred = da[None, :] + db[None, :] * tp[:, None]
    c0 = (Xb * inv_sig).astype(np.float32)
    c0 = np.where(msk, np.float32(SAT), c0).reshape(-1)
    qp = (pred * inv_sig).astype(np.float32)
    qp = np.where(msk, inv_sig, qp).reshape(-1)
    c016 = c0.astype(np.float16)
    q16 = qp.astype(np.float16)
    # final combine reads mask*obs and (1-mask) directly
    obsf = np.where(msk, obs, np.float32(0.0)).reshape(-1).astype(np.float32)
    maskf = (~msk).astype(np.float32).reshape(-1)

    eps16 = epsf.reshape(N, TF).astype(np.float16)
    in_maps = []
    for c in range(N_CORES):
        shard = np.ascontiguousarray(eps16[c * NLOC:(c + 1) * NLOC])
        in_maps.append(
            {"eps16": shard, "c016": c016, "q16": q16, "obs": obsf,
             "maskf": maskf, "ones": np.ones(P, np.float32)}
        )
    kr = run_bass_kernel_spmd(nc, in_maps, list(range(N_CORES)), trace=TRACE)
    LAST_RESULTS = kr
    return kr.results[0]["out"].reshape(T, F).astype(np.float32)
